# revision 1
# baseline (speedup 1.0000x reference)
"""Causal self-attention (B=4, T=2048, C=1024, H=16) on 8 trn2 NeuronCores.

Sharding: batch x head. Core c owns batch b=c//2 and heads hh*8..hh*8+8
(hh=c%2), i.e. 512 of the 1024 qkv channels for 2048 tokens. QKV is
column-parallel over the core's channels, attention is fully local per head,
the output projection is row-parallel over the core's channels and the two
partial outputs per batch are summed on the host (+ bp).

All device I/O is a single packed bf16 input tensor per core (x k-tiles +
weight slices + biases; masks and the transpose identity are generated
on-device) and one bf16 partial output [2048, 1024] per core. This minimizes
both per-call buffer-handle count and bytes, which dominate the axon
dispatch cost that the timed loop measures.

Device kernel (per core), all matmuls bf16 (psum f32):
  - qT/kT/vT = W_slice^T-form matmuls over x k-tiles, 4 channel groups of
    128 (= 2 heads) each -> SBUF resident bf16
  - S^T tiles [s=128, t<=512] per (head-group, head, s-tile), block-causal
  - P^T = exp(S^T/8) on ACT (no-max softmax; scores are O(6)) -> bf16,
    diagonal tiles masked in place by a gpsimd affine_select
  - P@V + softmax denominator in one bf16 matmul (ones column in V)
  - y = num * (1/den) on DVE (bf16), PE-transpose, Wp matmul accumulating
    the 4 head-groups in psum (superblocks 0-2, deferred as tail fill) or
    incrementally into SBUF f32 accumulators (superblock 3) -> partial
    output [2048, 1024] bf16 in DRAM
"""

import os
import sys

for _p in ("/opt/trn_rl_repo", "/root/.axon_site/_ro/trn_rl_repo"):
    if os.path.isdir(_p) and _p not in sys.path:
        sys.path.insert(0, _p)

import numpy as np

B, T, C = 4, 2048, 1024
H, D = 16, 64
N_CORES = 8
CH = 512                   # qkv channels per core (= 8 heads x 64)
HPC = 8                    # heads per core
HG = 4                     # head groups of 2 heads (128 channels)
P = 128
TB = T                     # tokens per core (its batch)
TSB = 512                  # token superblock
N_TSB = TB // TSB          # 4
ST_PER_B = TB // P         # 16 s-tiles
KT = C // P                # 8 contraction k-tiles

# packed input column layout (bf16, [128, NCOL])
XOFF = 0                   # [kt 8, tok 2048]
WQOFF = XOFF + KT * TB     # [kt 8, ch 512]
WKOFF = WQOFF + KT * CH
WVOFF = WKOFF + KT * CH
WPOFF = WVOFF + KT * CH    # [cg 4, out 1024]
BOFF = WPOFF + HG * C      # [kind 3, cg 4]
NCOL = BOFF + 12

_RUNNER = None
CFG = {'pm': 2, 'pmq': 2, 'ptr': 1, 'ppv': 1, 'pt': 28, 'la': 1,
       'up': 0, 'ytp': 66, 'yp': 5, 'op': 4, 'vt': 3,
       'cads': (1, 1.5, 2, 1.5)}


def _build_nc():
    import concourse.mybir as mybir
    import concourse.tile as tile
    from concourse import bacc

    f32 = mybir.dt.float32
    bf16 = mybir.dt.bfloat16
    MULT = mybir.AluOpType.mult
    ADD = mybir.AluOpType.add
    GE = mybir.AluOpType.is_ge
    EQ = mybir.AluOpType.is_equal
    EXP = mybir.ActivationFunctionType.Exp

    nc = bacc.Bacc("TRN2", target_bir_lowering=False, debug=False,
                   num_devices=N_CORES, enable_partition_id=False)

    pk = nc.dram_tensor("pk", [P, NCOL], bf16, kind="ExternalInput")
    outp = nc.dram_tensor("outp", [TB, C], bf16, kind="ExternalOutput")

    with tile.TileContext(nc) as tc:
        with (
            tc.tile_pool(name="const", bufs=1) as const,
            tc.tile_pool(name="big", bufs=1) as big,
            tc.tile_pool(name="xp", bufs=CFG.get('xp', 2)) as xp,
            tc.tile_pool(name="vt", bufs=CFG.get('vt', 2)) as vtp,
            tc.tile_pool(name="pt", bufs=CFG['pt']) as ptp,
            tc.tile_pool(name="yp", bufs=CFG.get('yp', 3)) as yp,
            tc.tile_pool(name="ytp", bufs=CFG.get('ytp', 24)) as ytp,
            tc.tile_pool(name="rp", bufs=4) as rp,
            tc.tile_pool(name="op", bufs=CFG.get('op', 3)) as op,
            tc.tile_pool(name="pm", bufs=CFG['pm'], space="PSUM") as pm,
            tc.tile_pool(name="pmq", bufs=CFG['pmq'], space="PSUM") as pmq,
            tc.tile_pool(name="ptr", bufs=CFG['ptr'], space="PSUM") as ptr,
            tc.tile_pool(name="ppv", bufs=CFG['ppv'], space="PSUM") as ppv,
        ):
            # ---- constants ----
            wq_sb = const.tile([P, HG, KT, P], bf16, tag="wq")
            wk_sb = const.tile([P, HG, KT, P], bf16, tag="wk")
            wv_sb = const.tile([P, HG, KT, P], bf16, tag="wv")
            wp_sb = const.tile([P, HG, C], bf16, tag="wp")
            b_bf = const.tile([P, 12], bf16, tag="bbf")
            b_f32 = const.tile([P, 12], f32, tag="bf32")
            identb_sb = const.tile([P, P], bf16, tag="identb")
            ones_sb = const.tile([P, P], bf16, tag="ones")

            src = pk.ap()

            wqsrc = src[:, WQOFF:WQOFF + KT * CH].rearrange(
                "p (g a m) -> p g a m", g=HG, a=KT)
            wksrc = src[:, WKOFF:WKOFF + KT * CH].rearrange(
                "p (g a m) -> p g a m", g=HG, a=KT)

            def emit_consts_q():
                # the first two k-tiles of wq channel group 0 and the first
                # x chunk alone gate the opening projection matmuls; they
                # are issued ahead of this (see prologue)
                nc.sync.dma_start(wq_sb[:, 0, 2:KT], wqsrc[:, 0, 2:KT])
                nc.sync.dma_start(b_bf[:], src[:, BOFF:BOFF + 12])
                nc.vector.tensor_copy(b_f32[:], b_bf[:])
                # on-device constants: identity (for PE transpose) and the
                # upper-triangular causal mask, via iota predicates (no DMA)
                nc.vector.memset(ones_sb[:], 1.0)
                nc.gpsimd.affine_select(
                    identb_sb[:], ones_sb[:], [[1, P]], EQ, 0.0,
                    base=0, channel_multiplier=-1)

            def emit_consts_k():
                nc.sync.dma_start(wk_sb[:, 0], wksrc[:, 0])

            def emit_consts_qk_rest():
                for cg in range(1, HG):
                    nc.sync.dma_start(wq_sb[:, cg], wqsrc[:, cg])
                    nc.sync.dma_start(wk_sb[:, cg], wksrc[:, cg])

            def emit_consts_v():
                nc.sync.dma_start(
                    wv_sb[:],
                    src[:, WVOFF:WVOFF + KT * CH].rearrange(
                        "p (g a m) -> p g a m", g=HG, a=KT))

            def emit_consts_rest():
                nc.sync.dma_start(
                    wp_sb[:],
                    src[:, WPOFF:WPOFF + HG * C].rearrange(
                        "p (a m) -> p a m", a=HG))

            # ---- resident activation buffers ----
            qT_sb = big.tile([P, HG, TB], bf16, tag="qT")
            kT_sb = big.tile([P, HG, TB], bf16, tag="kT")
            # v layout: per s-tile idx (16): 8 heads x 65 cols = [v (64)|ones]
            v_sb = big.tile([P, ST_PER_B * HPC * 65], bf16, tag="v")
            nc.vector.memset(v_sb[:, 64::65], 1.0)

            def emit_S_pair(sb, hg, st, pts):
                """Both heads' (hg, st) score tiles in one 2-bank psum, one
                exp instruction for the pair (+ diag masks)."""
                t0 = sb * TSB
                n0 = max(0, st - 4 * sb) * P
                ps = pm.tile([P, 2, TSB], f32, tag="mm", name="ps")
                for h in range(2):
                    lhs = kT_sb[h * 64:h * 64 + 64, hg, st * P:(st + 1) * P]
                    rhs = qT_sb[h * 64:h * 64 + 64, hg, t0 + n0:t0 + TSB]
                    nc.tensor.matmul(ps[:, h, n0:TSB], lhs, rhs, start=True,
                                     stop=True)
                ptile = ptp.tile([P, 2, TSB], bf16, tag="pt", name="ptile")
                nc.scalar.activation(
                    ptile[:, 0:2, n0:TSB], ps[:, 0:2, n0:TSB], EXP,
                    scale=0.125)
                if st >= 4 * sb:
                    # in-place block-causal mask: keep cols >= partition idx
                    for h in range(2):
                        nc.gpsimd.affine_select(
                            ptile[:, h, n0:n0 + P], ptile[:, h, n0:n0 + P],
                            [[1, P]], GE, 0.0, base=0, channel_multiplier=-1)
                pts[st] = ptile

            def attn_PV_unit(sb, hg, j, pts, yts):
                """Generator: PV + div + transpose for one (hg, j) block."""
                y_t = yp.tile([P, P], bf16, tag="y", name="y_t")
                nv = 4 * sb + j + 1
                for h in range(2):
                    pv = ppv.tile([P, 65], f32, tag="pv", name="pv")
                    for st in range(nv):
                        ptile = pts[st]
                        nc.tensor.matmul(
                            pv[:],
                            ptile[:, h, j * P:(j + 1) * P],
                            v_sb[:, st * HPC * 65 + (hg * 2 + h) * 65:
                                 st * HPC * 65 + (hg * 2 + h) * 65 + 65],
                            start=(st == 0), stop=(st == nv - 1))
                    rec = rp.tile([P, 1], f32, tag="rec", name="rec")
                    nc.vector.reciprocal(rec[:], pv[:, 64:65])
                    nc.vector.tensor_scalar_mul(
                        y_t[:, h * 64:(h + 1) * 64], pv[:, 0:64], rec[:, 0:1])
                pst = ptr.tile([P, P], bf16, tag="tr", name="pst")
                nc.tensor.transpose(pst[:], y_t[:], identb_sb[:])
                yt_t = ytp.tile([P, P], bf16, tag="yt", name="yt_t")
                nc.any.tensor_copy(yt_t[:], pst[:])
                yts[(sb, hg, j)] = yt_t
                yield

            def wp_unit(sb, j, yts):
                """Generator: Wp for output t-block (sb, j), all head groups."""
                ot = op.tile([P, C], bf16, tag="ot", name="ot")
                for half in range(2):
                    wps = pmq.tile([P, TSB], f32, tag="mmq", name="wps")
                    for hg in range(HG):
                        nc.tensor.matmul(
                            wps[:], yts[(sb, hg, j)],
                            wp_sb[:, hg, half * TSB:(half + 1) * TSB],
                            start=(hg == 0), stop=(hg == HG - 1))
                    nc.vector.tensor_copy(
                        ot[:, half * TSB:(half + 1) * TSB], wps[:])
                r = sb * 4 + j
                nc.sync.dma_start(outp.ap()[r * P:(r + 1) * P, :], ot[:])
                yield

            # sb3 output accumulators: Wp contributions land incrementally
            # per head group so the kernel tail only carries the last one
            acc_sb = big.tile([P, 4, C], f32, tag="acc")

            def wp_inc_unit(sb, hg, j, yts):
                """Generator: one head group's Wp contribution for (sb3, j).
                hg 0..2 accumulate in a f32 SBUF tile; hg 3 adds into a fresh
                bf16 tile (single rounding) that is DMA'd out."""
                ot = None
                if hg == HG - 1:
                    ot = op.tile([P, C], bf16, tag="ot", name="ot")
                for half in range(2):
                    wps = pmq.tile([P, TSB], f32, tag="mmq", name="wps")
                    nc.tensor.matmul(
                        wps[:], yts[(sb, hg, j)],
                        wp_sb[:, hg, half * TSB:(half + 1) * TSB],
                        start=True, stop=True)
                    dst = acc_sb[:, j, half * TSB:(half + 1) * TSB]
                    if hg == 0:
                        nc.vector.tensor_copy(dst, wps[:])
                    elif hg < HG - 1:
                        nc.vector.tensor_tensor(dst, dst, wps[:], ADD)
                    else:
                        nc.vector.tensor_tensor(
                            ot[:, half * TSB:(half + 1) * TSB], dst, wps[:],
                            ADD)
                        r = sb * 4 + j
                        nc.sync.dma_start(
                            outp.ap()[r * P:(r + 1) * P,
                                      half * TSB:(half + 1) * TSB],
                            ot[:, half * TSB:(half + 1) * TSB])
                yield

            xsrc = src[:, XOFF:XOFF + KT * TB].rearrange(
                "p (a t) -> p a t", a=KT)

            def emit_xt(tsb):
                xt = xp.tile([P, KT, TSB], bf16, tag="xt", name="xt")
                for c in range(4):  # chunked so matmuls start early
                    nc.sync.dma_start(
                        xt[:, 2 * c:2 * c + 2, :],
                        xsrc[:, 2 * c:2 * c + 2, tsb * TSB:(tsb + 1) * TSB])
                return xt

            def emit_proj(w_sb, bcol, kind, cg, xt, tsb):
                ps = pmq.tile([P, TSB], f32, tag="mmq", name="ps")
                for kt in range(KT):
                    nc.tensor.matmul(
                        ps[:],
                        w_sb[:, cg, kt, :],
                        xt[:, kt, :],
                        start=(kt == 0), stop=(kt == KT - 1),
                    )
                bsl = b_f32[:, bcol + cg:bcol + cg + 1]
                if kind == "q":
                    nc.vector.tensor_scalar_add(
                        qT_sb[:, cg, tsb * TSB:(tsb + 1) * TSB], ps[:], bsl)
                elif kind == "k":
                    nc.vector.tensor_scalar_add(
                        kT_sb[:, cg, tsb * TSB:(tsb + 1) * TSB], ps[:], bsl)
                else:
                    vt_t = vtp.tile([P, TSB], bf16, tag="vt", name="vt_t")
                    nc.vector.tensor_scalar_add(vt_t[:], ps[:], bsl)
                    for q4 in range(4):
                        pst = ptr.tile([P, P], bf16, tag="tr", name="pst")
                        nc.tensor.transpose(
                            pst[:], vt_t[:, q4 * P:(q4 + 1) * P],
                            identb_sb[:])
                        idx = tsb * 4 + q4
                        base = idx * HPC * 65 + (cg * 2) * 65
                        nc.vector.tensor_copy(
                            v_sb[:, base:base + 64], pst[:, 0:64])
                        nc.vector.tensor_copy(
                            v_sb[:, base + 65:base + 129], pst[:, 64:128])

            def qkv_units(tsb):
                """Generator: one (kind, cg) projection per unit."""
                xt = emit_xt(tsb)
                for (w_sb, bcol, kind) in (
                    (wq_sb, 0, "q"), (wk_sb, 4, "k"), (wv_sb, 8, "v"),
                ):
                    for cg in range(HG):
                        emit_proj(w_sb, bcol, kind, cg, xt, tsb)
                        yield

            # Fine-grained software-pipelined emission (see baseline): S
            # emission interleaves with draining PV/Wp units of earlier
            # blocks and QKV units of later superblocks as PE fill work.
            from collections import deque
            fill = deque()     # (kind, key, generator)

            def drain_one():
                while fill:
                    kind, key, g = fill[0]
                    try:
                        next(g)
                        return True
                    except StopIteration:
                        fill.popleft()
                return False

            def force_drain_qkv(max_tsb):
                for ent in list(fill):
                    kind, key, g = ent
                    if kind == "qkv" and key <= max_tsb:
                        for _ in g:
                            pass
                        fill.remove(ent)

            queued = set()

            def queue_qkv_upto(tsb_max):
                for t in range(tsb_max + 1):
                    if t not in queued:
                        queued.add(t)
                        fill.append(("qkv", t, qkv_units(t)))

            cads = CFG.get('cads', (1, 1, 1, 2))
            la = CFG.get('la', 2)
            yts = {}
            wp_defer = []
            k = 0

            # hand-scheduled startup: channel-chunked const DMAs interleave
            # with qkv(0), and sb0's S tiles slot in right after each head
            # group's q/k projections so the PE never waits on the full
            # weight stream
            nc.sync.dma_start(wq_sb[:, 0, 0:2], wqsrc[:, 0, 0:2])
            xt0 = emit_xt(0)
            emit_consts_q()
            queued.add(0)
            emit_consts_k()
            emit_consts_qk_rest()
            pts0 = [{} for _ in range(HG)]
            for cg in range(HG):
                emit_proj(wq_sb, 0, "q", cg, xt0, 0)
                emit_proj(wk_sb, 4, "k", cg, xt0, 0)
                if cg == 0:
                    emit_consts_v()
                for st in range(4):
                    emit_S_pair(0, cg, st, pts0[cg])
                    fill.append(("pv", (0, cg, st),
                                 attn_PV_unit(0, cg, st, pts0[cg], yts)))
            for cg in range(HG):
                emit_proj(wv_sb, 8, "v", cg, xt0, 0)
            emit_consts_rest()
            for j in range(4):
                wp_defer.append(("wp", (0, j), wp_unit(0, j, yts)))
            for sb in range(1, N_TSB):
                queue_qkv_upto(min(max(sb + la, sb), N_TSB - 1))
                force_drain_qkv(sb)
                if sb == N_TSB - 1:
                    # Wp units of earlier superblocks were held back as PE
                    # fill for this ACT-bound phase (most S tiles, no QKV)
                    fill.extend(wp_defer)
                    wp_defer.clear()
                cad = cads[sb]
                credit = 0.0
                for hg in range(HG):
                    pts = {}
                    nst = 4 * sb + 4
                    for st in range(nst):
                        emit_S_pair(sb, hg, st, pts)
                        k += 1
                        credit += 1.0 / cad
                        while credit >= 1.0:
                            credit -= 1.0
                            drain_one()
                        j = st - 4 * sb
                        if 0 <= j <= 3:
                            fill.append(
                                ("pv", (sb, hg, j),
                                 attn_PV_unit(sb, hg, j, pts, yts)))
                            if sb == N_TSB - 1:
                                fill.append(
                                    ("wpi", (sb, hg, j),
                                     wp_inc_unit(sb, hg, j, yts)))
                            elif hg == HG - 1:
                                wp_defer.append(
                                    ("wp", (sb, j), wp_unit(sb, j, yts)))
            while drain_one():
                pass

    nc.compile()
    return nc


class _Runner:
    """Compiles the Bass module once and exposes a sharded 8-core callable."""

    def __init__(self):
        import jax
        import jax.numpy as jnp  # noqa: F401
        from jax.sharding import Mesh, PartitionSpec
        from jax.experimental.shard_map import shard_map
        import concourse.mybir as mybir
        from concourse import bass2jax

        self.jax = jax
        nc = _build_nc()
        self.nc = nc
        bass2jax.install_neuronx_cc_hook()

        partition_name = (nc.partition_id_tensor.name
                          if nc.partition_id_tensor else None)
        in_names, out_names, out_avals, zero_shapes = [], [], [], []
        for alloc in nc.m.functions[0].allocations:
            if not isinstance(alloc, mybir.MemoryLocationSet):
                continue
            name = alloc.memorylocations[0].name
            if alloc.kind == "ExternalInput":
                if name != partition_name:
                    in_names.append(name)
            elif alloc.kind == "ExternalOutput":
                out_names.append(name)
                shape = tuple(alloc.tensor_shape)
                dtype = mybir.dt.np(alloc.dtype)
                out_avals.append(jax.core.ShapedArray(shape, dtype))
                zero_shapes.append((shape, dtype))
        self.in_names = list(in_names)
        self.out_names = list(out_names)
        self.zero_shapes = zero_shapes
        n_params = len(in_names)
        n_outs = len(out_names)
        all_in_names = in_names + out_names
        if partition_name is not None:
            all_in_names = all_in_names + [partition_name]

        def _body(*args):
            operands = list(args)
            if partition_name is not None:
                operands.append(bass2jax.partition_id_tensor())
            outs = bass2jax._bass_exec_p.bind(
                *operands,
                out_avals=tuple(out_avals),
                in_names=tuple(all_in_names),
                out_names=tuple(out_names),
                lowering_input_output_aliases=(),
                sim_require_finite=True,
                sim_require_nnan=True,
                nc=nc,
            )
            return tuple(outs)

        devices = jax.devices()[:N_CORES]
        mesh = Mesh(np.asarray(devices), ("core",))
        self.mesh = mesh
        self.spec = PartitionSpec("core")
        donate = tuple(range(n_params, n_params + n_outs))
        self.sharded = jax.jit(
            shard_map(
                _body, mesh=mesh,
                in_specs=(PartitionSpec("core"),) * (n_params + n_outs),
                out_specs=(PartitionSpec("core"),) * n_outs,
                check_rep=False,
            ),
            donate_argnums=donate,
            keep_unused=True,
        )

    def make_zero_outs(self):
        return [np.zeros((N_CORES * s[0], *s[1:]), d)
                for s, d in self.zero_shapes]

    def run(self, concat_inputs):
        out_arrs = self.sharded(*concat_inputs, *self.make_zero_outs())
        return [np.asarray(a) for a in out_arrs]


def _get_runner():
    global _RUNNER
    if _RUNNER is None:
        _RUNNER = _Runner()
    return _RUNNER


def prep_inputs(x, Wq, bq, Wk, bk, Wv, bv, Wp, bp):
    """Build the concatenated (axis-0 stacked over cores) device inputs."""
    import ml_dtypes
    bf = ml_dtypes.bfloat16
    x = np.asarray(x, np.float32).reshape(B, T, C)
    Wq = np.asarray(Wq, np.float32)
    Wk = np.asarray(Wk, np.float32)
    Wv = np.asarray(Wv, np.float32)
    Wp = np.asarray(Wp, np.float32)
    bq = np.asarray(bq, np.float32)
    bk = np.asarray(bk, np.float32)
    bv = np.asarray(bv, np.float32)

    def ktiles(a):  # [rows=KT*P, cols] -> [P, KT*cols]
        r, c = a.shape
        n = r // P
        return np.ascontiguousarray(
            a.reshape(n, P, c).transpose(1, 0, 2).reshape(P, n * c))

    def cgtiles(a):  # [K=KT*P, CH=HG*P] -> [P, HG*KT*P] (cg-major)
        return np.ascontiguousarray(
            a.reshape(KT, P, HG, P).transpose(1, 2, 0, 3).reshape(P, -1))

    per_core = {"pk": []}
    for i in range(N_CORES):
        b = i // 2
        hh = i % 2
        cs = slice(hh * CH, (hh + 1) * CH)
        xr = ktiles(x[b].T)                                   # [P, 8*2048]
        wqr = cgtiles(np.ascontiguousarray(Wq[cs, :].T))      # [P, 4*8*128]
        wkr = cgtiles(np.ascontiguousarray(Wk[cs, :].T))
        wvr = cgtiles(np.ascontiguousarray(Wv[cs, :].T))
        wpr = ktiles(np.ascontiguousarray(Wp[:, cs].T))       # [P, 4*1024]
        bias = np.concatenate(
            [bv_[cs].reshape(HG, P).T for bv_ in (bq, bk, bv)],
            axis=1)                                           # [P, 12]
        pkr = np.concatenate([xr, wqr, wkr, wvr, wpr, bias],
                             axis=1).astype(bf)
        assert pkr.shape == (P, NCOL), pkr.shape
        per_core["pk"].append(pkr)
    return per_core


def kernel(x, Wq, bq, Wk, bk, Wv, bv, Wp, bp):
    runner = _get_runner()
    per_core = prep_inputs(x, Wq, bq, Wk, bk, Wv, bv, Wp, bp)
    concat_in = [np.concatenate(per_core[n], axis=0) for n in runner.in_names]
    outs = runner.run(concat_in)
    # single output: per-core partials [8 * TB, C]; cores 2b, 2b+1 hold the
    # two halves of batch b's row-parallel output projection
    partials = outs[0].astype(np.float32).reshape(N_CORES, TB, C)
    bp = np.asarray(bp, np.float32)
    out = np.empty((B, T, C), np.float32)
    for b in range(B):
        out[b] = partials[2 * b] + partials[2 * b + 1] + bp[None, :]
    return out



# revision 3
# speedup vs baseline: 5.5296x; 5.5296x over previous
"""Causal self-attention (B=4, T=2048, C=1024, H=16) on 8 trn2 NeuronCores.

Sharding: batch x head. Core c owns batch b=c//2 and heads hh*8..hh*8+8
(hh=c%2), i.e. 512 of the 1024 qkv channels for 2048 tokens. QKV is
column-parallel over the core's channels, attention is fully local per head,
the output projection is row-parallel over the core's channels and the two
partial outputs per batch are summed on the host (+ bp).

All device I/O is a single packed bf16 input tensor per core (x k-tiles +
weight slices + biases; masks and the transpose identity are generated
on-device) and one bf16 partial output [2048, 1024] per core. This minimizes
both per-call buffer-handle count and bytes, which dominate the axon
dispatch cost that the timed loop measures.

Device kernel (per core), all matmuls bf16 (psum f32):
  - qT/kT/vT = W_slice^T-form matmuls over x k-tiles, 4 channel groups of
    128 (= 2 heads) each -> SBUF resident bf16
  - S^T tiles [s=128, t<=512] per (head-group, head, s-tile), block-causal
  - P^T = exp(S^T/8) on ACT (no-max softmax; scores are O(6)) -> bf16,
    diagonal tiles masked in place by a gpsimd affine_select
  - P@V + softmax denominator in one bf16 matmul (ones column in V)
  - y = num * (1/den) on DVE (bf16), PE-transpose, Wp matmul accumulating
    the 4 head-groups in psum (superblocks 0-2, deferred as tail fill) or
    incrementally into SBUF f32 accumulators (superblock 3) -> partial
    output [2048, 1024] bf16 in DRAM
"""

import os
import sys

for _p in ("/opt/trn_rl_repo", "/root/.axon_site/_ro/trn_rl_repo"):
    if os.path.isdir(_p) and _p not in sys.path:
        sys.path.insert(0, _p)

import numpy as np

B, T, C = 4, 2048, 1024
H, D = 16, 64
N_CORES = 8
CH = 512                   # qkv channels per core (= 8 heads x 64)
HPC = 8                    # heads per core
HG = 4                     # head groups of 2 heads (128 channels)
P = 128
TB = T                     # tokens per core (its batch)
TSB = 512                  # token superblock
N_TSB = TB // TSB          # 4
ST_PER_B = TB // P         # 16 s-tiles
KT = C // P                # 8 contraction k-tiles

# packed input column layout (bf16, [128, NCOL])
XOFF = 0                   # [kt 8, tok 2048]
WQOFF = XOFF + KT * TB     # [kt 8, ch 512]
WKOFF = WQOFF + KT * CH
WVOFF = WKOFF + KT * CH
WPOFF = WVOFF + KT * CH    # [cg 4, out 1024]
BOFF = WPOFF + HG * C      # [kind 3, cg 4]
NCOL = BOFF + 12

_RUNNER = None
CFG = {'pm': 2, 'pmq': 2, 'ptr': 1, 'ppv': 1, 'pt': 28, 'la': 1,
       'up': 0, 'ytp': 66, 'yp': 5, 'op': 4, 'vt': 3,
       'cads': (1, 1.5, 2, 1.5)}


def _build_nc():
    import concourse.mybir as mybir
    import concourse.tile as tile
    from concourse import bacc

    f32 = mybir.dt.float32
    bf16 = mybir.dt.bfloat16
    MULT = mybir.AluOpType.mult
    ADD = mybir.AluOpType.add
    GE = mybir.AluOpType.is_ge
    EQ = mybir.AluOpType.is_equal
    EXP = mybir.ActivationFunctionType.Exp

    nc = bacc.Bacc("TRN2", target_bir_lowering=False, debug=False,
                   num_devices=N_CORES, enable_partition_id=False)

    pk = nc.dram_tensor("pk", [P, NCOL], bf16, kind="ExternalInput")
    outp = nc.dram_tensor("outp", [TB, C], bf16, kind="ExternalOutput")

    with tile.TileContext(nc) as tc:
        with (
            tc.tile_pool(name="const", bufs=1) as const,
            tc.tile_pool(name="big", bufs=1) as big,
            tc.tile_pool(name="xp", bufs=CFG.get('xp', 2)) as xp,
            tc.tile_pool(name="vt", bufs=CFG.get('vt', 2)) as vtp,
            tc.tile_pool(name="pt", bufs=CFG['pt']) as ptp,
            tc.tile_pool(name="yp", bufs=CFG.get('yp', 3)) as yp,
            tc.tile_pool(name="ytp", bufs=CFG.get('ytp', 24)) as ytp,
            tc.tile_pool(name="rp", bufs=4) as rp,
            tc.tile_pool(name="op", bufs=CFG.get('op', 3)) as op,
            tc.tile_pool(name="pm", bufs=CFG['pm'], space="PSUM") as pm,
            tc.tile_pool(name="pmq", bufs=CFG['pmq'], space="PSUM") as pmq,
            tc.tile_pool(name="ptr", bufs=CFG['ptr'], space="PSUM") as ptr,
            tc.tile_pool(name="ppv", bufs=CFG['ppv'], space="PSUM") as ppv,
        ):
            # ---- constants ----
            wq_sb = const.tile([P, HG, KT, P], bf16, tag="wq")
            wk_sb = const.tile([P, HG, KT, P], bf16, tag="wk")
            wv_sb = const.tile([P, HG, KT, P], bf16, tag="wv")
            wp_sb = const.tile([P, HG, C], bf16, tag="wp")
            b_bf = const.tile([P, 12], bf16, tag="bbf")
            b_f32 = const.tile([P, 12], f32, tag="bf32")
            identb_sb = const.tile([P, P], bf16, tag="identb")
            ones_sb = const.tile([P, P], bf16, tag="ones")

            src = pk.ap()

            wqsrc = src[:, WQOFF:WQOFF + KT * CH].rearrange(
                "p (g a m) -> p g a m", g=HG, a=KT)
            wksrc = src[:, WKOFF:WKOFF + KT * CH].rearrange(
                "p (g a m) -> p g a m", g=HG, a=KT)

            def emit_consts_q():
                # the first two k-tiles of wq channel group 0 and the first
                # x chunk alone gate the opening projection matmuls; they
                # are issued ahead of this (see prologue)
                nc.sync.dma_start(wq_sb[:, 0, 2:KT], wqsrc[:, 0, 2:KT])
                nc.sync.dma_start(b_bf[:], src[:, BOFF:BOFF + 12])
                nc.vector.tensor_copy(b_f32[:], b_bf[:])
                # on-device constants: identity (for PE transpose) and the
                # upper-triangular causal mask, via iota predicates (no DMA)
                nc.vector.memset(ones_sb[:], 1.0)
                nc.gpsimd.affine_select(
                    identb_sb[:], ones_sb[:], [[1, P]], EQ, 0.0,
                    base=0, channel_multiplier=-1)

            def emit_consts_k():
                nc.sync.dma_start(wk_sb[:, 0], wksrc[:, 0])

            def emit_consts_qk_rest():
                for cg in range(1, HG):
                    nc.sync.dma_start(wq_sb[:, cg], wqsrc[:, cg])
                    nc.sync.dma_start(wk_sb[:, cg], wksrc[:, cg])

            def emit_consts_v():
                nc.sync.dma_start(
                    wv_sb[:],
                    src[:, WVOFF:WVOFF + KT * CH].rearrange(
                        "p (g a m) -> p g a m", g=HG, a=KT))

            def emit_consts_rest():
                nc.sync.dma_start(
                    wp_sb[:],
                    src[:, WPOFF:WPOFF + HG * C].rearrange(
                        "p (a m) -> p a m", a=HG))

            # ---- resident activation buffers ----
            qT_sb = big.tile([P, HG, TB], bf16, tag="qT")
            kT_sb = big.tile([P, HG, TB], bf16, tag="kT")
            # v layout: per s-tile idx (16): 8 heads x 65 cols = [v (64)|ones]
            v_sb = big.tile([P, ST_PER_B * HPC * 65], bf16, tag="v")
            nc.vector.memset(v_sb[:, 64::65], 1.0)

            def emit_S_pair(sb, hg, st, pts):
                """Both heads' (hg, st) score tiles in one 2-bank psum, one
                exp instruction for the pair (+ diag masks)."""
                t0 = sb * TSB
                n0 = max(0, st - 4 * sb) * P
                ps = pm.tile([P, 2, TSB], f32, tag="mm", name="ps")
                for h in range(2):
                    lhs = kT_sb[h * 64:h * 64 + 64, hg, st * P:(st + 1) * P]
                    rhs = qT_sb[h * 64:h * 64 + 64, hg, t0 + n0:t0 + TSB]
                    nc.tensor.matmul(ps[:, h, n0:TSB], lhs, rhs, start=True,
                                     stop=True)
                ptile = ptp.tile([P, 2, TSB], bf16, tag="pt", name="ptile")
                nc.scalar.activation(
                    ptile[:, 0:2, n0:TSB], ps[:, 0:2, n0:TSB], EXP,
                    scale=0.125)
                if st >= 4 * sb:
                    # in-place block-causal mask: keep cols >= partition idx
                    for h in range(2):
                        nc.gpsimd.affine_select(
                            ptile[:, h, n0:n0 + P], ptile[:, h, n0:n0 + P],
                            [[1, P]], GE, 0.0, base=0, channel_multiplier=-1)
                pts[st] = ptile

            def attn_PV_unit(sb, hg, j, pts, yts):
                """Generator: PV + div + transpose for one (hg, j) block."""
                y_t = yp.tile([P, P], bf16, tag="y", name="y_t")
                nv = 4 * sb + j + 1
                for h in range(2):
                    pv = ppv.tile([P, 65], f32, tag="pv", name="pv")
                    for st in range(nv):
                        ptile = pts[st]
                        nc.tensor.matmul(
                            pv[:],
                            ptile[:, h, j * P:(j + 1) * P],
                            v_sb[:, st * HPC * 65 + (hg * 2 + h) * 65:
                                 st * HPC * 65 + (hg * 2 + h) * 65 + 65],
                            start=(st == 0), stop=(st == nv - 1))
                    rec = rp.tile([P, 1], f32, tag="rec", name="rec")
                    nc.vector.reciprocal(rec[:], pv[:, 64:65])
                    nc.vector.tensor_scalar_mul(
                        y_t[:, h * 64:(h + 1) * 64], pv[:, 0:64], rec[:, 0:1])
                pst = ptr.tile([P, P], bf16, tag="tr", name="pst")
                nc.tensor.transpose(pst[:], y_t[:], identb_sb[:])
                yt_t = ytp.tile([P, P], bf16, tag="yt", name="yt_t")
                nc.any.tensor_copy(yt_t[:], pst[:])
                yts[(sb, hg, j)] = yt_t
                yield

            def wp_unit(sb, j, yts):
                """Generator: Wp for output t-block (sb, j), all head groups."""
                ot = op.tile([P, C], bf16, tag="ot", name="ot")
                for half in range(2):
                    wps = pmq.tile([P, TSB], f32, tag="mmq", name="wps")
                    for hg in range(HG):
                        nc.tensor.matmul(
                            wps[:], yts[(sb, hg, j)],
                            wp_sb[:, hg, half * TSB:(half + 1) * TSB],
                            start=(hg == 0), stop=(hg == HG - 1))
                    nc.vector.tensor_copy(
                        ot[:, half * TSB:(half + 1) * TSB], wps[:])
                r = sb * 4 + j
                nc.sync.dma_start(outp.ap()[r * P:(r + 1) * P, :], ot[:])
                yield

            # sb3 output accumulators: Wp contributions land incrementally
            # per head group so the kernel tail only carries the last one
            acc_sb = big.tile([P, 4, C], f32, tag="acc")

            def wp_inc_unit(sb, hg, j, yts):
                """Generator: one head group's Wp contribution for (sb3, j).
                hg 0..2 accumulate in a f32 SBUF tile; hg 3 adds into a fresh
                bf16 tile (single rounding) that is DMA'd out."""
                ot = None
                if hg == HG - 1:
                    ot = op.tile([P, C], bf16, tag="ot", name="ot")
                for half in range(2):
                    wps = pmq.tile([P, TSB], f32, tag="mmq", name="wps")
                    nc.tensor.matmul(
                        wps[:], yts[(sb, hg, j)],
                        wp_sb[:, hg, half * TSB:(half + 1) * TSB],
                        start=True, stop=True)
                    dst = acc_sb[:, j, half * TSB:(half + 1) * TSB]
                    if hg == 0:
                        nc.vector.tensor_copy(dst, wps[:])
                    elif hg < HG - 1:
                        nc.vector.tensor_tensor(dst, dst, wps[:], ADD)
                    else:
                        nc.vector.tensor_tensor(
                            ot[:, half * TSB:(half + 1) * TSB], dst, wps[:],
                            ADD)
                        r = sb * 4 + j
                        nc.sync.dma_start(
                            outp.ap()[r * P:(r + 1) * P,
                                      half * TSB:(half + 1) * TSB],
                            ot[:, half * TSB:(half + 1) * TSB])
                yield

            xsrc = src[:, XOFF:XOFF + KT * TB].rearrange(
                "p (a t) -> p a t", a=KT)

            def emit_xt(tsb):
                xt = xp.tile([P, KT, TSB], bf16, tag="xt", name="xt")
                for c in range(4):  # chunked so matmuls start early
                    nc.sync.dma_start(
                        xt[:, 2 * c:2 * c + 2, :],
                        xsrc[:, 2 * c:2 * c + 2, tsb * TSB:(tsb + 1) * TSB])
                return xt

            def emit_proj(w_sb, bcol, kind, cg, xt, tsb):
                ps = pmq.tile([P, TSB], f32, tag="mmq", name="ps")
                for kt in range(KT):
                    nc.tensor.matmul(
                        ps[:],
                        w_sb[:, cg, kt, :],
                        xt[:, kt, :],
                        start=(kt == 0), stop=(kt == KT - 1),
                    )
                bsl = b_f32[:, bcol + cg:bcol + cg + 1]
                if kind == "q":
                    nc.vector.tensor_scalar_add(
                        qT_sb[:, cg, tsb * TSB:(tsb + 1) * TSB], ps[:], bsl)
                elif kind == "k":
                    nc.vector.tensor_scalar_add(
                        kT_sb[:, cg, tsb * TSB:(tsb + 1) * TSB], ps[:], bsl)
                else:
                    vt_t = vtp.tile([P, TSB], bf16, tag="vt", name="vt_t")
                    nc.vector.tensor_scalar_add(vt_t[:], ps[:], bsl)
                    for q4 in range(4):
                        pst = ptr.tile([P, P], bf16, tag="tr", name="pst")
                        nc.tensor.transpose(
                            pst[:], vt_t[:, q4 * P:(q4 + 1) * P],
                            identb_sb[:])
                        idx = tsb * 4 + q4
                        base = idx * HPC * 65 + (cg * 2) * 65
                        nc.vector.tensor_copy(
                            v_sb[:, base:base + 64], pst[:, 0:64])
                        nc.vector.tensor_copy(
                            v_sb[:, base + 65:base + 129], pst[:, 64:128])

            def qkv_units(tsb):
                """Generator: one (kind, cg) projection per unit."""
                xt = emit_xt(tsb)
                for (w_sb, bcol, kind) in (
                    (wq_sb, 0, "q"), (wk_sb, 4, "k"), (wv_sb, 8, "v"),
                ):
                    for cg in range(HG):
                        emit_proj(w_sb, bcol, kind, cg, xt, tsb)
                        yield

            # Fine-grained software-pipelined emission (see baseline): S
            # emission interleaves with draining PV/Wp units of earlier
            # blocks and QKV units of later superblocks as PE fill work.
            from collections import deque
            fill = deque()     # (kind, key, generator)

            def drain_one():
                while fill:
                    kind, key, g = fill[0]
                    try:
                        next(g)
                        return True
                    except StopIteration:
                        fill.popleft()
                return False

            def force_drain_qkv(max_tsb):
                for ent in list(fill):
                    kind, key, g = ent
                    if kind == "qkv" and key <= max_tsb:
                        for _ in g:
                            pass
                        fill.remove(ent)

            queued = set()

            def queue_qkv_upto(tsb_max):
                for t in range(tsb_max + 1):
                    if t not in queued:
                        queued.add(t)
                        fill.append(("qkv", t, qkv_units(t)))

            cads = CFG.get('cads', (1, 1, 1, 2))
            la = CFG.get('la', 2)
            yts = {}
            wp_defer = []
            k = 0

            # hand-scheduled startup: channel-chunked const DMAs interleave
            # with qkv(0), and sb0's S tiles slot in right after each head
            # group's q/k projections so the PE never waits on the full
            # weight stream
            nc.sync.dma_start(wq_sb[:, 0, 0:2], wqsrc[:, 0, 0:2])
            xt0 = emit_xt(0)
            emit_consts_q()
            queued.add(0)
            emit_consts_k()
            emit_consts_qk_rest()
            pts0 = [{} for _ in range(HG)]
            for cg in range(HG):
                emit_proj(wq_sb, 0, "q", cg, xt0, 0)
                emit_proj(wk_sb, 4, "k", cg, xt0, 0)
                if cg == 0:
                    emit_consts_v()
                for st in range(4):
                    emit_S_pair(0, cg, st, pts0[cg])
                    fill.append(("pv", (0, cg, st),
                                 attn_PV_unit(0, cg, st, pts0[cg], yts)))
            for cg in range(HG):
                emit_proj(wv_sb, 8, "v", cg, xt0, 0)
            emit_consts_rest()
            for j in range(4):
                wp_defer.append(("wp", (0, j), wp_unit(0, j, yts)))
            for sb in range(1, N_TSB):
                queue_qkv_upto(min(max(sb + la, sb), N_TSB - 1))
                force_drain_qkv(sb)
                if sb == N_TSB - 1:
                    # Wp units of earlier superblocks were held back as PE
                    # fill for this ACT-bound phase (most S tiles, no QKV)
                    fill.extend(wp_defer)
                    wp_defer.clear()
                cad = cads[sb]
                credit = 0.0
                for hg in range(HG):
                    pts = {}
                    nst = 4 * sb + 4
                    for st in range(nst):
                        emit_S_pair(sb, hg, st, pts)
                        k += 1
                        credit += 1.0 / cad
                        while credit >= 1.0:
                            credit -= 1.0
                            drain_one()
                        j = st - 4 * sb
                        if 0 <= j <= 3:
                            fill.append(
                                ("pv", (sb, hg, j),
                                 attn_PV_unit(sb, hg, j, pts, yts)))
                            if sb == N_TSB - 1:
                                fill.append(
                                    ("wpi", (sb, hg, j),
                                     wp_inc_unit(sb, hg, j, yts)))
                            elif hg == HG - 1:
                                wp_defer.append(
                                    ("wp", (sb, j), wp_unit(sb, j, yts)))
            while drain_one():
                pass

    nc.compile()
    return nc


class _Runner:
    """Compiles the Bass module once and exposes a sharded 8-core callable."""

    def __init__(self):
        import jax
        import jax.numpy as jnp  # noqa: F401
        from jax.sharding import Mesh, PartitionSpec
        from jax.experimental.shard_map import shard_map
        import concourse.mybir as mybir
        from concourse import bass2jax

        self.jax = jax
        nc = _build_nc()
        self.nc = nc
        bass2jax.install_neuronx_cc_hook()

        partition_name = (nc.partition_id_tensor.name
                          if nc.partition_id_tensor else None)
        in_names, out_names, out_avals, zero_shapes = [], [], [], []
        for alloc in nc.m.functions[0].allocations:
            if not isinstance(alloc, mybir.MemoryLocationSet):
                continue
            name = alloc.memorylocations[0].name
            if alloc.kind == "ExternalInput":
                if name != partition_name:
                    in_names.append(name)
            elif alloc.kind == "ExternalOutput":
                out_names.append(name)
                shape = tuple(alloc.tensor_shape)
                dtype = mybir.dt.np(alloc.dtype)
                out_avals.append(jax.core.ShapedArray(shape, dtype))
                zero_shapes.append((shape, dtype))
        self.in_names = list(in_names)
        self.out_names = list(out_names)
        self.zero_shapes = zero_shapes
        n_params = len(in_names)
        n_outs = len(out_names)
        all_in_names = in_names + out_names
        if partition_name is not None:
            all_in_names = all_in_names + [partition_name]

        def _body(*args):
            operands = list(args)
            if partition_name is not None:
                operands.append(bass2jax.partition_id_tensor())
            outs = bass2jax._bass_exec_p.bind(
                *operands,
                out_avals=tuple(out_avals),
                in_names=tuple(all_in_names),
                out_names=tuple(out_names),
                lowering_input_output_aliases=(),
                sim_require_finite=True,
                sim_require_nnan=True,
                nc=nc,
            )
            return tuple(outs)

        devices = jax.devices()[:N_CORES]
        mesh = Mesh(np.asarray(devices), ("core",))
        self.mesh = mesh
        self.spec = PartitionSpec("core")
        donate = tuple(range(n_params, n_params + n_outs))

        # fast-dispatch AOT compile: suppresses the bass_effect so calls take
        # jax's C++ fast path (the effectful path adds ~350us of host work
        # per call, which is serial with the axon dispatch stream)
        from jax.sharding import NamedSharding
        shard = NamedSharding(mesh, self.spec)
        in_avals = []
        for alloc in nc.m.functions[0].allocations:
            if not isinstance(alloc, mybir.MemoryLocationSet):
                continue
            name = alloc.memorylocations[0].name
            if alloc.kind == "ExternalInput" and name != partition_name:
                shape = tuple(alloc.tensor_shape)
                in_avals.append(jax.ShapeDtypeStruct(
                    (N_CORES * shape[0], *shape[1:]), mybir.dt.np(alloc.dtype),
                    sharding=shard))
        out_zero_avals = [
            jax.ShapeDtypeStruct((N_CORES * s[0], *s[1:]), d, sharding=shard)
            for s, d in zero_shapes]

        def _compile():
            return jax.jit(
                shard_map(
                    _body, mesh=mesh,
                    in_specs=(PartitionSpec("core"),) * (n_params + n_outs),
                    out_specs=(PartitionSpec("core"),) * n_outs,
                    check_rep=False,
                ),
                donate_argnums=donate,
                keep_unused=True,
            ).lower(*in_avals, *out_zero_avals).compile()

        self.sharded = bass2jax.fast_dispatch_compile(_compile)
        self.shard = shard

    def make_zero_outs(self):
        return [self.jax.device_put(np.zeros((N_CORES * s[0], *s[1:]), d),
                                    self.shard)
                for s, d in self.zero_shapes]

    def run(self, concat_inputs):
        concat_inputs = [self.jax.device_put(a, self.shard)
                         for a in concat_inputs]
        out_arrs = self.sharded(*concat_inputs, *self.make_zero_outs())
        return [np.asarray(a) for a in out_arrs]


def _get_runner():
    global _RUNNER
    if _RUNNER is None:
        _RUNNER = _Runner()
    return _RUNNER


def prep_inputs(x, Wq, bq, Wk, bk, Wv, bv, Wp, bp):
    """Build the concatenated (axis-0 stacked over cores) device inputs."""
    import ml_dtypes
    bf = ml_dtypes.bfloat16
    x = np.asarray(x, np.float32).reshape(B, T, C)
    Wq = np.asarray(Wq, np.float32)
    Wk = np.asarray(Wk, np.float32)
    Wv = np.asarray(Wv, np.float32)
    Wp = np.asarray(Wp, np.float32)
    bq = np.asarray(bq, np.float32)
    bk = np.asarray(bk, np.float32)
    bv = np.asarray(bv, np.float32)

    def ktiles(a):  # [rows=KT*P, cols] -> [P, KT*cols]
        r, c = a.shape
        n = r // P
        return np.ascontiguousarray(
            a.reshape(n, P, c).transpose(1, 0, 2).reshape(P, n * c))

    def cgtiles(a):  # [K=KT*P, CH=HG*P] -> [P, HG*KT*P] (cg-major)
        return np.ascontiguousarray(
            a.reshape(KT, P, HG, P).transpose(1, 2, 0, 3).reshape(P, -1))

    per_core = {"pk": []}
    for i in range(N_CORES):
        b = i // 2
        hh = i % 2
        cs = slice(hh * CH, (hh + 1) * CH)
        xr = ktiles(x[b].T)                                   # [P, 8*2048]
        wqr = cgtiles(np.ascontiguousarray(Wq[cs, :].T))      # [P, 4*8*128]
        wkr = cgtiles(np.ascontiguousarray(Wk[cs, :].T))
        wvr = cgtiles(np.ascontiguousarray(Wv[cs, :].T))
        wpr = ktiles(np.ascontiguousarray(Wp[:, cs].T))       # [P, 4*1024]
        bias = np.concatenate(
            [bv_[cs].reshape(HG, P).T for bv_ in (bq, bk, bv)],
            axis=1)                                           # [P, 12]
        pkr = np.concatenate([xr, wqr, wkr, wvr, wpr, bias],
                             axis=1).astype(bf)
        assert pkr.shape == (P, NCOL), pkr.shape
        per_core["pk"].append(pkr)
    return per_core


def kernel(x, Wq, bq, Wk, bk, Wv, bv, Wp, bp):
    runner = _get_runner()
    per_core = prep_inputs(x, Wq, bq, Wk, bk, Wv, bv, Wp, bp)
    concat_in = [np.concatenate(per_core[n], axis=0) for n in runner.in_names]
    outs = runner.run(concat_in)
    # single output: per-core partials [8 * TB, C]; cores 2b, 2b+1 hold the
    # two halves of batch b's row-parallel output projection
    partials = outs[0].astype(np.float32).reshape(N_CORES, TB, C)
    bp = np.asarray(bp, np.float32)
    out = np.empty((B, T, C), np.float32)
    for b in range(B):
        out[b] = partials[2 * b] + partials[2 * b + 1] + bp[None, :]
    return out

